# revision 3
# baseline (speedup 1.0000x reference)
"""RGCN (2-layer, per-(dst,rel) mean aggregation) + triplet projection,
distributed over 8 Trainium2 NeuronCores (one SPMD Bass/Tile program).

Sharding: destination-node ranges (6250 nodes/core). Aggregate-first:
  y[dst,rel] = (1/cnt) * sum_{src} x[src]   built as one-hot "slab" matmuls
  (slab[e, dst%128] = 1/cnt, accumulated in PSUM over 128-edge chunks), then
  agg[n,:] = sum_r y_r[n,:] @ W_r + x[n,:] @ root + b, ReLU -> AllGather h.
Triplet: u = h@Wp[:256]+bp, v = h@Wp[256:] per node, AllGather u & v, then
  out[e] = u[src_e] + v[dst_e] via dma_gather + vector add.

The instruction stream is identical on all cores: per-(window,rel,region) edge
runs are padded to a cross-core-common chunk skeleton; per-core variation lives
in the gather-index / slab input tensors. dma_gather's int16 index limit
(32767 < 50000 rows) is handled by splitting edges into lo (src<32768, table
base row 0) and hi (src>=32768, table base row 17232) streams.
"""

import numpy as np
import ml_dtypes

BF16 = ml_dtypes.bfloat16

N, R, F, E, NCORES = 50000, 8, 256, 400000, 8
NC = N // NCORES             # 6250
W = (NC + 127) // 128        # 49 windows/core
NPAD = W * 128               # 6272
SPLIT = 32768
HIBASE = 17232               # hi idx = src - HIBASE (<= 32767)
PC = 16                      # gather piece = 16 chunks = 2048 rows
LAST_EXEC_NS = None
LAST_TRACE = None


def _wrap_idx(idx):
    """int32 indices -> [128, nchunk]: chunk c's 128 rows on partitions."""
    return np.ascontiguousarray(idx.reshape(-1, 128).T)


def _plan_agg(src, dst, et, norm):
    core = dst // NC
    percore = []
    counts = np.zeros((NCORES, W, R, 2), dtype=np.int64)
    for c in range(NCORES):
        m = np.where(core == c)[0]
        dl = dst[m] - c * NC
        w = dl >> 7
        reg = (src[m] >= SPLIT).astype(np.int64)
        percore.append((m, dl, w, reg))
        key = (w * R + et[m]) * 2 + reg
        counts[c] = np.bincount(key, minlength=W * R * 2).reshape(W, R, 2)
    K = np.maximum(1, -(-counts.max(axis=0) // 128))   # [W,R,2] chunks
    chunk_of = np.zeros((W, R, 2), dtype=np.int64)
    nchunk = [0, 0]
    for reg in range(2):
        acc = 0
        for w in range(W):
            for r in range(R):
                chunk_of[w, r, reg] = acc
                acc += K[w, r, reg]
        nchunk[reg] = acc
    idx_s, slab_s = [], []
    for c in range(NCORES):
        m, dl, w, regs = percore[c]
        r = et[m]
        ipair, spair = [], []
        for reg in range(2):
            slots = nchunk[reg] * 128
            idx = np.zeros(slots, dtype=np.int32)
            slab = np.zeros((slots, 128), dtype=np.float32)
            sel = regs == reg
            mm = m[sel]
            order = np.lexsort((dl[sel], r[sel] + R * w[sel]))
            mm = mm[order]
            wsel, rsel, dsel = w[sel][order], r[sel][order], dl[sel][order]
            runkey = wsel * R + rsel
            runstart = chunk_of[wsel, rsel, reg] * 128
            off = np.arange(len(mm))
            starts = np.zeros(len(mm), dtype=np.int64)
            b = np.flatnonzero(np.diff(runkey)) + 1
            starts[b] = off[b]
            starts = np.maximum.accumulate(starts)
            pos = runstart + (off - starts)
            s = src[mm]
            idx[pos] = s
            slab[pos, dsel & 127] = norm[mm]
            ipair.append(idx)
            spair.append(slab.astype(BF16))
        idx_s.append(ipair)
        slab_s.append(spair)
    return K, chunk_of, nchunk, idx_s, slab_s


def _plan_trip(src, dst):
    EC = E // NCORES
    gsizes = np.zeros((NCORES, 4), dtype=np.int64)
    percore = []
    for c in range(NCORES):
        ids = np.arange(c * EC, (c + 1) * EC)
        g = (src[ids] >= SPLIT) * 2 + (dst[ids] >= SPLIT)
        order = np.argsort(g, kind="stable")
        ids, g = ids[order], g[order]
        percore.append((ids, g))
        gsizes[c] = np.bincount(g, minlength=4)
    gpad = (-(-gsizes.max(axis=0) // 128)) * 128
    gstart = np.concatenate([[0], np.cumsum(gpad)]).astype(np.int64)
    slots = int(gstart[-1])
    plans = []
    for c in range(NCORES):
        ids, g = percore[c]
        iu = np.zeros(slots, dtype=np.int32)
        iv = np.zeros(slots, dtype=np.int32)
        orig = np.full(slots, -1, dtype=np.int64)
        for gi in range(4):
            sel = ids[g == gi]
            a = int(gstart[gi])
            s, d = src[sel], dst[sel]
            iu[a:a + len(sel)] = s
            iv[a:a + len(sel)] = d
            orig[a:a + len(sel)] = sel
        plans.append((iu, iv, orig))
    return gstart, slots, plans


def _build(nchunk, K, chunk_of, gstart, tslots):
    import concourse.bass as bass
    import concourse.bacc as bacc
    import concourse.mybir as mybir
    import concourse.tile as tile

    dt = mybir.dt
    nc = bacc.Bacc("TRN2", target_bir_lowering=False, debug=False,
                   num_devices=NCORES)
    AF = mybir.ActivationFunctionType

    x16 = nc.dram_tensor("x16", [N, F], dt.bfloat16, kind="ExternalInput")
    xsh = nc.dram_tensor("xsh", [NPAD, F], dt.bfloat16, kind="ExternalInput")
    w1d = nc.dram_tensor("w1", [R, F, F], dt.bfloat16, kind="ExternalInput")
    w2d = nc.dram_tensor("w2", [R, F, F], dt.bfloat16, kind="ExternalInput")
    r1d = nc.dram_tensor("r1", [F, F], dt.bfloat16, kind="ExternalInput")
    r2d = nc.dram_tensor("r2", [F, F], dt.bfloat16, kind="ExternalInput")
    b1d = nc.dram_tensor("b1", [128, F], dt.float32, kind="ExternalInput")
    b2d = nc.dram_tensor("b2", [128, F], dt.float32, kind="ExternalInput")
    wpud = nc.dram_tensor("wpu", [F, F], dt.bfloat16, kind="ExternalInput")
    wpvd = nc.dram_tensor("wpv", [F, F], dt.bfloat16, kind="ExternalInput")
    bpd = nc.dram_tensor("bp", [128, F], dt.float32, kind="ExternalInput")
    ilo_d = nc.dram_tensor("idx_lo", [128, nchunk[0]], dt.int32, kind="ExternalInput")
    ihi_d = nc.dram_tensor("idx_hi", [128, nchunk[1]], dt.int32, kind="ExternalInput")
    slo_d = nc.dram_tensor("slab_lo", [nchunk[0] * 128, 128], dt.bfloat16, kind="ExternalInput")
    shi_d = nc.dram_tensor("slab_hi", [nchunk[1] * 128, 128], dt.bfloat16, kind="ExternalInput")
    tui_d = nc.dram_tensor("tui", [128, tslots // 128], dt.int32, kind="ExternalInput")
    tvi_d = nc.dram_tensor("tvi", [128, tslots // 128], dt.int32, kind="ExternalInput")
    tout = nc.dram_tensor("tout", [tslots, F], dt.bfloat16, kind="ExternalOutput")

    rg = [list(range(NCORES))]

    with tile.TileContext(nc) as tc:
        with (
            tc.tile_pool(name="const", bufs=1) as cp,
            tc.tile_pool(name="msg", bufs=2) as msgp,
            tc.tile_pool(name="slab", bufs=2) as slabp,
            tc.tile_pool(name="yw", bufs=2) as yp,
            tc.tile_pool(name="small", bufs=4) as sp,
            tc.tile_pool(name="ps", bufs=1, space="PSUM") as psp,
            tc.tile_pool(name="psagg", bufs=1, space="PSUM") as psaggp,
            tc.tile_pool(name="dram", bufs=1, space="DRAM") as dram,
        ):
            w_sb = [cp.tile([128, 16, F], dt.bfloat16, tag=f"w{i}", name=f"w{i}") for i in range(2)]
            nc.sync.dma_start(w_sb[0][:], w1d.ap().rearrange("r (h p) o -> p (r h) o", p=128))
            nc.sync.dma_start(w_sb[1][:], w2d.ap().rearrange("r (h p) o -> p (r h) o", p=128))
            rt_sb = [cp.tile([128, 2, F], dt.bfloat16, tag=f"rt{i}", name=f"rt{i}") for i in range(2)]
            nc.sync.dma_start(rt_sb[0][:], r1d.ap().rearrange("(h p) o -> p h o", p=128))
            nc.sync.dma_start(rt_sb[1][:], r2d.ap().rearrange("(h p) o -> p h o", p=128))
            b_sb = [cp.tile([128, F], dt.float32, tag=f"b{i}", name=f"b{i}") for i in range(2)]
            nc.sync.dma_start(b_sb[0][:], b1d[:])
            nc.sync.dma_start(b_sb[1][:], b2d[:])
            wpu_sb = cp.tile([128, 2, F], dt.bfloat16, tag="wpu", name="wpu")
            wpv_sb = cp.tile([128, 2, F], dt.bfloat16, tag="wpv", name="wpv")
            nc.sync.dma_start(wpu_sb[:], wpud.ap().rearrange("(h p) o -> p h o", p=128))
            nc.sync.dma_start(wpv_sb[:], wpvd.ap().rearrange("(h p) o -> p h o", p=128))
            bp_sb = cp.tile([128, F], dt.float32, tag="bp", name="bp")
            nc.sync.dma_start(bp_sb[:], bpd[:])
            ilo_sb = cp.tile([128, nchunk[0]], dt.int32, tag="ilo", name="ilo")
            ihi_sb = cp.tile([128, nchunk[1]], dt.int32, tag="ihi", name="ihi")
            nc.sync.dma_start(ilo_sb[:], ilo_d[:])
            nc.sync.dma_start(ihi_sb[:], ihi_d[:])
            tui_sb = cp.tile([128, tslots // 128], dt.int32, tag="tui", name="tui")
            tvi_sb = cp.tile([128, tslots // 128], dt.int32, tag="tvi", name="tvi")
            nc.sync.dma_start(tui_sb[:], tui_d[:])
            nc.sync.dma_start(tvi_sb[:], tvi_d[:])

            h1b = dram.tile([NPAD, F], dt.bfloat16, tag="h1b", name="h1b")
            h2b = dram.tile([NPAD, F], dt.bfloat16, tag="h2b", name="h2b")
            h1f = dram.tile([N, F], dt.bfloat16, addr_space="Shared", tag="h1f", name="h1f")
            uvb = [dram.tile([NPAD, F], dt.bfloat16, tag=f"uvb{i}", name=f"uvb{i}") for i in range(2)]
            uvf = [dram.tile([N, F], dt.bfloat16, addr_space="Shared", tag=f"uvf{i}", name=f"uvf{i}")
                   for i in range(2)]

            def gather_piece(table, idx_sb, c0, nblk, tag):
                """Gather chunks [c0, c0+nblk) of a stream into an SBUF tile."""
                t = msgp.tile([128, nblk, F], dt.bfloat16, tag=tag, name=tag)
                for b in range(nblk):
                    nc.gpsimd.indirect_dma_start(
                        out=t[:, b, :], out_offset=None, in_=table,
                        in_offset=bass.IndirectOffsetOnAxis(
                            ap=idx_sb[:, c0 + b:c0 + b + 1], axis=0))
                return t

            def layer(li, table, rootsrc, hout):
                idx_sb = (ilo_sb, ihi_sb)
                slab_d = (slo_d, shi_d)
                pieces = [{}, {}]   # region -> piece idx -> (msg_tile, slab_tile)

                def get_piece(reg, p):
                    if p not in pieces[reg]:
                        nblk = min(PC, nchunk[reg] - p * PC)
                        mt = gather_piece(table, idx_sb[reg], p * PC, nblk,
                                          f"m{reg}")
                        st = slabp.tile([128, nblk, 128], dt.bfloat16, tag=f"s{reg}", name=f"s{reg}")
                        nc.sync.dma_start(
                            st[:], slab_d[reg][p * PC * 128:(p * PC + nblk) * 128, :]
                            .rearrange("(b p) n -> p b n", p=128))
                        pieces[reg] = {p: (mt, st)}  # keep only latest
                    return pieces[reg][p]

                for w in range(W):
                    ps = [[psp.tile([128, 512], dt.float32, tag=f"ps{fh}{q}", name=f"ps{fh}{q}")
                           for q in range(2)] for fh in range(2)]
                    for r in range(R):
                        for reg in range(2):
                            for k in range(int(K[w, r, reg])):
                                ch = int(chunk_of[w, r, reg]) + k
                                p, b = divmod(ch, PC)
                                mt, st = get_piece(reg, p)
                                for fh in range(2):
                                    nc.tensor.matmul(
                                        ps[fh][r // 4][:, (r % 4) * 128:(r % 4) * 128 + 128],
                                        lhsT=mt[:, b, fh * 128:(fh + 1) * 128],
                                        rhs=st[:, b, :],
                                        start=(reg == 0 and k == 0),
                                        stop=(reg == 1 and k == int(K[w, r, 1]) - 1),
                                    )
                    yw = yp.tile([128, 2048], dt.bfloat16, tag="yw", name="yw")
                    for fh in range(2):
                        for q in range(2):
                            eng = nc.vector if q == 0 else nc.scalar
                            (eng.tensor_copy if q == 0 else eng.copy)(
                                yw[:, (fh * 2 + q) * 512:(fh * 2 + q + 1) * 512],
                                ps[fh][q][:])
                    xt = sp.tile([128, 2, 128], dt.bfloat16, tag="xt", name="xt")
                    for fh in range(2):
                        nc.sync.dma_start(
                            xt[:, fh, :],
                            rootsrc[w * 128:(w + 1) * 128, fh * 128:(fh + 1) * 128],
                            transpose=True)
                    agg = psaggp.tile([128, F], dt.float32, tag="agg", name="agg")
                    for r in range(R):
                        for fh in range(2):
                            nc.tensor.matmul(
                                agg[:], lhsT=yw[:, (fh * 8 + r) * 128:(fh * 8 + r + 1) * 128],
                                rhs=w_sb[li][:, r * 2 + fh, :],
                                start=(r == 0 and fh == 0), stop=False)
                    for fh in range(2):
                        nc.tensor.matmul(agg[:], lhsT=xt[:, fh, :],
                                         rhs=rt_sb[li][:, fh, :],
                                         start=False, stop=(fh == 1))
                    hf = sp.tile([128, F], dt.float32, tag="hf", name="hf")
                    nc.vector.tensor_tensor(hf[:], agg[:],
                                            b_sb[li][:],
                                            op=mybir.AluOpType.add)
                    hw = sp.tile([128, F], dt.bfloat16, tag="hw", name="hw")
                    nc.scalar.activation(hw[:], hf[:], AF.Relu)
                    nc.sync.dma_start(hout[w * 128:(w + 1) * 128, :], hw[:])

            layer(0, x16.ap(), xsh.ap(), h1b)
            nc.gpsimd.collective_compute(
                "AllGather", mybir.AluOpType.bypass, replica_groups=rg,
                ins=[h1b[0:NC, :].opt()], outs=[h1f[:].opt()])
            layer(1, h1f[:], h1b[:], h2b)

            # triplet projections u, v per node tile
            for w in range(W):
                ht = sp.tile([128, 2, 128], dt.bfloat16, tag="ht", name="ht")
                for fh in range(2):
                    nc.sync.dma_start(
                        ht[:, fh, :],
                        h2b[w * 128:(w + 1) * 128, fh * 128:(fh + 1) * 128],
                        transpose=True)
                psu = psaggp.tile([128, F], dt.float32, tag="psu", name="psu")
                psv = psaggp.tile([128, F], dt.float32, tag="psv", name="psv")
                for fh in range(2):
                    nc.tensor.matmul(psu[:], lhsT=ht[:, fh, :], rhs=wpu_sb[:, fh, :],
                                     start=(fh == 0), stop=(fh == 1))
                    nc.tensor.matmul(psv[:], lhsT=ht[:, fh, :], rhs=wpv_sb[:, fh, :],
                                     start=(fh == 0), stop=(fh == 1))
                uo = sp.tile([128, F], dt.bfloat16, tag="uo", name="uo")
                nc.vector.tensor_tensor(uo[:], psu[:],
                                        bp_sb[:],
                                        op=mybir.AluOpType.add)
                vo = sp.tile([128, F], dt.bfloat16, tag="vo", name="vo")
                nc.scalar.copy(vo[:], psv[:])
                nc.sync.dma_start(uvb[0][w * 128:(w + 1) * 128, :], uo[:])
                nc.sync.dma_start(uvb[1][w * 128:(w + 1) * 128, :], vo[:])
            for i in range(2):
                nc.gpsimd.collective_compute(
                    "AllGather", mybir.AluOpType.bypass, replica_groups=rg,
                    ins=[uvb[i][0:NC, :].opt()], outs=[uvf[i][:].opt()])

            # triplet gather + add, piece by piece (pieces stay inside groups)
            for gi in range(4):
                a = int(gstart[gi]) // 128
                nb = (int(gstart[gi + 1]) - int(gstart[gi])) // 128
                ub = uvf[0][:]
                vb = uvf[1][:]
                for p0 in range(0, nb, PC):
                    blks = min(PC, nb - p0)
                    gu = gather_piece(ub, tui_sb, a + p0, blks, "gu")
                    gv = gather_piece(vb, tvi_sb, a + p0, blks, "gv")
                    ot = msgp.tile([128, blks, F], dt.bfloat16, tag="ot", name="ot")
                    nc.vector.tensor_tensor(
                        ot[:].rearrange("p b o -> p (b o)"),
                        gu[:].rearrange("p b o -> p (b o)"),
                        gv[:].rearrange("p b o -> p (b o)"),
                        op=mybir.AluOpType.add)
                    nc.sync.dma_start(
                        tout[(a + p0) * 128:(a + p0 + blks) * 128, :]
                        .rearrange("(b p) o -> p b o", p=128),
                        ot[:])
    nc.compile()
    return nc


def kernel(**inputs):
    from concourse.bass_utils import run_bass_kernel_spmd

    x = np.asarray(inputs["x"], dtype=np.float32)
    ei = np.asarray(inputs["edge_index"], dtype=np.int64)
    et = np.asarray(inputs["edge_type"], dtype=np.int64)
    src, dst = ei[0], ei[1]
    cnt = np.bincount(dst * R + et, minlength=N * R)
    norm = (1.0 / np.maximum(cnt[dst * R + et], 1)).astype(np.float32)

    K, chunk_of, nchunk, idx_s, slab_s = _plan_agg(src, dst, et, norm)
    gstart, tslots, tplans = _plan_trip(src, dst)
    nc = _build(nchunk, K, chunk_of, gstart, tslots)

    x16 = x.astype(BF16)
    xpad = np.zeros((NPAD, F), dtype=BF16)
    w1 = np.asarray(inputs["W1"], np.float32).astype(BF16)
    w2 = np.asarray(inputs["W2"], np.float32).astype(BF16)
    r1 = np.asarray(inputs["root1"], np.float32).astype(BF16)
    r2 = np.asarray(inputs["root2"], np.float32).astype(BF16)
    wp = np.asarray(inputs["Wp"], np.float32)
    b1 = np.tile(np.asarray(inputs["b1"], np.float32).reshape(1, F), (128, 1))
    b2 = np.tile(np.asarray(inputs["b2"], np.float32).reshape(1, F), (128, 1))
    bp = np.tile(np.asarray(inputs["bp"], np.float32).reshape(1, F), (128, 1))

    in_maps = []
    for c in range(NCORES):
        xs = xpad.copy()
        xs[:NC] = x16[c * NC:(c + 1) * NC]
        iu, iv, _ = tplans[c]
        in_maps.append({
            "x16": x16, "xsh": xs,
            "w1": w1, "w2": w2, "r1": r1, "r2": r2,
            "b1": b1, "b2": b2,
            "wpu": wp[:F].astype(BF16), "wpv": wp[F:].astype(BF16), "bp": bp,
            "idx_lo": _wrap_idx(idx_s[c][0]), "idx_hi": _wrap_idx(idx_s[c][1]),
            "slab_lo": slab_s[c][0], "slab_hi": slab_s[c][1],
            "tui": _wrap_idx(iu), "tvi": _wrap_idx(iv),
        })
    import os
    res = None
    if os.environ.get("BASS_KERNEL_TRACE"):
        try:
            res = run_bass_kernel_spmd(nc, in_maps,
                                       core_ids=list(range(NCORES)), trace=True)
        except Exception:
            res = None
    if res is None:
        res = run_bass_kernel_spmd(nc, in_maps, core_ids=list(range(NCORES)))
    global LAST_EXEC_NS, LAST_TRACE
    LAST_EXEC_NS = res.exec_time_ns
    if res.instructions_and_trace is not None:
        LAST_TRACE = res.instructions_and_trace[1]
    out = np.zeros((E, F), dtype=np.float32)
    for c in range(NCORES):
        t = np.asarray(res.results[c]["tout"]).astype(np.float32)
        orig = tplans[c][2]
        valid = orig >= 0
        out[orig[valid]] = t[valid]
    return out



# revision 4
# speedup vs baseline: 1.4120x; 1.4120x over previous
"""RGCN (2-layer, per-(dst,rel) mean aggregation) + triplet projection,
distributed over 8 Trainium2 NeuronCores (one SPMD Bass/Tile program).

Sharding: destination-node ranges (6250 nodes/core). Aggregate-first:
  y[dst,rel] = (1/cnt) * sum_{src} x[src]   built as one-hot "slab" matmuls
  (slab[e, dst%128] = 1/cnt, accumulated in PSUM over 128-edge chunks), then
  agg[n,:] = sum_r y_r[n,:] @ W_r + x[n,:] @ root + b, ReLU -> AllGather h.
Triplet: u = h@Wp[:256]+bp, v = h@Wp[256:] per node, AllGather u & v, then
  out[e] = u[src_e] + v[dst_e] via dma_gather + vector add.

Gathers use gpsimd dma_gather (one instruction per 1024 rows; int16 indices,
amortizing the ~1us SWDGE fixed cost). The int16 limit (32767 < 50000 rows)
is handled by splitting edges into lo (src<32768, table base row 0) and hi
(src>=32768, table base row 17232) streams. One-hot slabs are built on-chip
((iota==dl)*norm on the vector engine) instead of streaming them from HBM.

The instruction stream is identical on all cores: per-(window,rel,region) edge
runs are padded to a cross-core-common chunk skeleton; per-core variation
lives in the gather-index / dl / norm input tensors.
"""

import numpy as np
import ml_dtypes

BF16 = ml_dtypes.bfloat16

N, R, F, E, NCORES = 50000, 8, 256, 400000, 8
NC = N // NCORES             # 6250
W = (NC + 127) // 128        # 49 windows/core
NPAD = W * 128               # 6272
SPLIT = 32768
HIBASE = 17232               # hi idx = src - HIBASE (<= 32767)
PC = 8                       # gather piece = 8 chunks = 1024 rows
LAST_EXEC_NS = None
LAST_TRACE = None


def _wrap_idx(idx):
    """int/float array [slots] -> [128, nchunk]: chunk c's 128 rows on
    partitions (slot = c*128 + p)."""
    return np.ascontiguousarray(idx.reshape(-1, 128).T)


def _wrap16(idx):
    """int16 idx [slots] -> [128, slots//16]: element i at [i%16, i//16],
    replicated across the 8 gpsimd core partition groups."""
    s = len(idx) // 16
    a = np.ascontiguousarray(idx.reshape(s, 16).T)
    return np.tile(a, (8, 1))


def _plan_agg(src, dst, et, norm):
    core = dst // NC
    percore = []
    counts = np.zeros((NCORES, W, R, 2), dtype=np.int64)
    for c in range(NCORES):
        m = np.where(core == c)[0]
        dl = dst[m] - c * NC
        w = dl >> 7
        reg = (src[m] >= SPLIT).astype(np.int64)
        percore.append((m, dl, w, reg))
        key = (w * R + et[m]) * 2 + reg
        counts[c] = np.bincount(key, minlength=W * R * 2).reshape(W, R, 2)
    K = np.maximum(1, -(-counts.max(axis=0) // 128))   # [W,R,2] chunks
    chunk_of = np.zeros((W, R, 2), dtype=np.int64)
    nchunk = [0, 0]
    for reg in range(2):
        acc = 0
        for w in range(W):
            for r in range(R):
                chunk_of[w, r, reg] = acc
                acc += K[w, r, reg]
        nchunk[reg] = acc
    idx_s, dl_s, nm_s = [], [], []
    for c in range(NCORES):
        m, dl, w, regs = percore[c]
        r = et[m]
        ipair, dpair, npair = [], [], []
        for reg in range(2):
            slots = nchunk[reg] * 128
            idx = np.zeros(slots, dtype=np.int32)
            dlv = np.zeros(slots, dtype=np.int32)
            nmv = np.zeros(slots, dtype=np.float32)
            sel = regs == reg
            mm = m[sel]
            order = np.lexsort((dl[sel], r[sel] + R * w[sel]))
            mm = mm[order]
            wsel, rsel, dsel = w[sel][order], r[sel][order], dl[sel][order]
            runkey = wsel * R + rsel
            runstart = chunk_of[wsel, rsel, reg] * 128
            off = np.arange(len(mm))
            starts = np.zeros(len(mm), dtype=np.int64)
            b = np.flatnonzero(np.diff(runkey)) + 1
            starts[b] = off[b]
            starts = np.maximum.accumulate(starts)
            pos = runstart + (off - starts)
            s = src[mm]
            idx[pos] = s - reg * HIBASE
            dlv[pos] = dsel & 127
            nmv[pos] = norm[mm]
            ipair.append(_wrap16(idx.astype(np.int16)))
            dpair.append(_wrap_idx(dlv.astype(BF16)))
            npair.append(_wrap_idx(nmv.astype(BF16)))
        idx_s.append(ipair)
        dl_s.append(dpair)
        nm_s.append(npair)
    return K, chunk_of, nchunk, idx_s, dl_s, nm_s


def _plan_trip(src, dst):
    EC = E // NCORES
    gsizes = np.zeros((NCORES, 4), dtype=np.int64)
    percore = []
    for c in range(NCORES):
        ids = np.arange(c * EC, (c + 1) * EC)
        g = (src[ids] >= SPLIT) * 2 + (dst[ids] >= SPLIT)
        order = np.argsort(g, kind="stable")
        ids, g = ids[order], g[order]
        percore.append((ids, g))
        gsizes[c] = np.bincount(g, minlength=4)
    gpad = (-(-gsizes.max(axis=0) // 128)) * 128
    gstart = np.concatenate([[0], np.cumsum(gpad)]).astype(np.int64)
    slots = int(gstart[-1])
    plans = []
    for c in range(NCORES):
        ids, g = percore[c]
        iu = np.zeros(slots, dtype=np.int32)
        iv = np.zeros(slots, dtype=np.int32)
        orig = np.full(slots, -1, dtype=np.int64)
        for gi in range(4):
            sel = ids[g == gi]
            a = int(gstart[gi])
            s, d = src[sel], dst[sel]
            iu[a:a + len(sel)] = s - (gi >> 1) * HIBASE
            iv[a:a + len(sel)] = d - (gi & 1) * HIBASE
            orig[a:a + len(sel)] = sel
        plans.append((_wrap16(iu.astype(np.int16)),
                      _wrap16(iv.astype(np.int16)), orig))
    return gstart, slots, plans


def _build(nchunk, K, chunk_of, gstart, tslots):
    import concourse.bass as bass
    import concourse.bacc as bacc
    import concourse.mybir as mybir
    import concourse.tile as tile

    dt = mybir.dt
    nc = bacc.Bacc("TRN2", target_bir_lowering=False, debug=False,
                   num_devices=NCORES)
    AF = mybir.ActivationFunctionType

    x16 = nc.dram_tensor("x16", [N, F], dt.bfloat16, kind="ExternalInput")
    xsh = nc.dram_tensor("xsh", [NPAD, F], dt.bfloat16, kind="ExternalInput")
    w1d = nc.dram_tensor("w1", [R, F, F], dt.bfloat16, kind="ExternalInput")
    w2d = nc.dram_tensor("w2", [R, F, F], dt.bfloat16, kind="ExternalInput")
    r1d = nc.dram_tensor("r1", [F, F], dt.bfloat16, kind="ExternalInput")
    r2d = nc.dram_tensor("r2", [F, F], dt.bfloat16, kind="ExternalInput")
    b1d = nc.dram_tensor("b1", [128, F], dt.float32, kind="ExternalInput")
    b2d = nc.dram_tensor("b2", [128, F], dt.float32, kind="ExternalInput")
    wpud = nc.dram_tensor("wpu", [F, F], dt.bfloat16, kind="ExternalInput")
    wpvd = nc.dram_tensor("wpv", [F, F], dt.bfloat16, kind="ExternalInput")
    bpd = nc.dram_tensor("bp", [128, F], dt.float32, kind="ExternalInput")
    ilo_d = nc.dram_tensor("idx_lo", [128, nchunk[0] * 8], dt.int16, kind="ExternalInput")
    ihi_d = nc.dram_tensor("idx_hi", [128, nchunk[1] * 8], dt.int16, kind="ExternalInput")
    dlo_d = nc.dram_tensor("dl_lo", [128, nchunk[0]], dt.bfloat16, kind="ExternalInput")
    dhi_d = nc.dram_tensor("dl_hi", [128, nchunk[1]], dt.bfloat16, kind="ExternalInput")
    nlo_d = nc.dram_tensor("nm_lo", [128, nchunk[0]], dt.bfloat16, kind="ExternalInput")
    nhi_d = nc.dram_tensor("nm_hi", [128, nchunk[1]], dt.bfloat16, kind="ExternalInput")
    tui_d = nc.dram_tensor("tui", [128, tslots // 16], dt.int16, kind="ExternalInput")
    tvi_d = nc.dram_tensor("tvi", [128, tslots // 16], dt.int16, kind="ExternalInput")
    tout = nc.dram_tensor("tout", [tslots, F], dt.bfloat16, kind="ExternalOutput")

    rg = [list(range(NCORES))]

    with tile.TileContext(nc) as tc:
        with (
            tc.tile_pool(name="const", bufs=1) as cp,
            tc.tile_pool(name="msg", bufs=3) as msgp,
            tc.tile_pool(name="slab", bufs=3) as slabp,
            tc.tile_pool(name="yw", bufs=2) as yp,
            tc.tile_pool(name="small", bufs=4) as sp,
            tc.tile_pool(name="ps", bufs=1, space="PSUM") as psp,
            tc.tile_pool(name="psagg", bufs=1, space="PSUM") as psaggp,
            tc.tile_pool(name="dram", bufs=1, space="DRAM") as dram,
        ):
            w_sb = [cp.tile([128, 16, F], dt.bfloat16, tag=f"w{i}", name=f"w{i}") for i in range(2)]
            nc.sync.dma_start(w_sb[0][:], w1d.ap().rearrange("r (h p) o -> p (r h) o", p=128))
            nc.sync.dma_start(w_sb[1][:], w2d.ap().rearrange("r (h p) o -> p (r h) o", p=128))
            rt_sb = [cp.tile([128, 2, F], dt.bfloat16, tag=f"rt{i}", name=f"rt{i}") for i in range(2)]
            nc.sync.dma_start(rt_sb[0][:], r1d.ap().rearrange("(h p) o -> p h o", p=128))
            nc.sync.dma_start(rt_sb[1][:], r2d.ap().rearrange("(h p) o -> p h o", p=128))
            b_sb = [cp.tile([128, F], dt.float32, tag=f"b{i}", name=f"b{i}") for i in range(2)]
            nc.sync.dma_start(b_sb[0][:], b1d[:])
            nc.sync.dma_start(b_sb[1][:], b2d[:])
            wpu_sb = cp.tile([128, 2, F], dt.bfloat16, tag="wpu", name="wpu")
            wpv_sb = cp.tile([128, 2, F], dt.bfloat16, tag="wpv", name="wpv")
            nc.sync.dma_start(wpu_sb[:], wpud.ap().rearrange("(h p) o -> p h o", p=128))
            nc.sync.dma_start(wpv_sb[:], wpvd.ap().rearrange("(h p) o -> p h o", p=128))
            bp_sb = cp.tile([128, F], dt.float32, tag="bp", name="bp")
            nc.sync.dma_start(bp_sb[:], bpd[:])
            ilo_sb = cp.tile([128, nchunk[0] * 8], dt.int16, tag="ilo", name="ilo")
            ihi_sb = cp.tile([128, nchunk[1] * 8], dt.int16, tag="ihi", name="ihi")
            nc.sync.dma_start(ilo_sb[:], ilo_d[:])
            nc.sync.dma_start(ihi_sb[:], ihi_d[:])
            dlo_sb = cp.tile([128, nchunk[0]], dt.bfloat16, tag="dlo", name="dlo")
            dhi_sb = cp.tile([128, nchunk[1]], dt.bfloat16, tag="dhi", name="dhi")
            nc.sync.dma_start(dlo_sb[:], dlo_d[:])
            nc.sync.dma_start(dhi_sb[:], dhi_d[:])
            nlo_sb = cp.tile([128, nchunk[0]], dt.bfloat16, tag="nlo", name="nlo")
            nhi_sb = cp.tile([128, nchunk[1]], dt.bfloat16, tag="nhi", name="nhi")
            nc.sync.dma_start(nlo_sb[:], nlo_d[:])
            nc.sync.dma_start(nhi_sb[:], nhi_d[:])
            tui_sb = cp.tile([128, tslots // 16], dt.int16, tag="tui", name="tui")
            tvi_sb = cp.tile([128, tslots // 16], dt.int16, tag="tvi", name="tvi")
            nc.sync.dma_start(tui_sb[:], tui_d[:])
            nc.sync.dma_start(tvi_sb[:], tvi_d[:])
            iota_sb = cp.tile([128, 128], dt.bfloat16, tag="iota", name="iota")
            nc.gpsimd.iota(iota_sb[:], pattern=[[1, 128]], channel_multiplier=0,
                           allow_small_or_imprecise_dtypes=True)

            h1b = dram.tile([NPAD, F], dt.bfloat16, tag="h1b", name="h1b")
            h2b = dram.tile([NPAD, F], dt.bfloat16, tag="h2b", name="h2b")
            h1f = dram.tile([N, F], dt.bfloat16, addr_space="Shared", tag="h1f", name="h1f")
            uvb = [dram.tile([NPAD, F], dt.bfloat16, tag=f"uvb{i}", name=f"uvb{i}") for i in range(2)]
            uvf = [dram.tile([N, F], dt.bfloat16, addr_space="Shared", tag=f"uvf{i}", name=f"uvf{i}")
                   for i in range(2)]

            def gather_piece(table, idx_sb, c0, nblk, tag):
                """Gather chunks [c0, c0+nblk) of a stream into an SBUF tile
                with one dma_gather (1024 rows max)."""
                t = msgp.tile([128, nblk, F], dt.bfloat16, tag=tag, name=tag)
                ni = nblk * 128
                nc.gpsimd.dma_gather(
                    t[:], table, idx_sb[:, c0 * 8:(c0 + nblk) * 8],
                    ni, ni, F)
                return t

            def layer(li, tables, rootsrc, hout):
                idx_sb = (ilo_sb, ihi_sb)
                dl_sb = (dlo_sb, dhi_sb)
                nm_sb = (nlo_sb, nhi_sb)
                pieces = [{}, {}]   # region -> piece idx -> (msg_tile, slab_tile)

                def get_piece(reg, p):
                    if p not in pieces[reg]:
                        nblk = min(PC, nchunk[reg] - p * PC)
                        mt = gather_piece(tables[reg], idx_sb[reg], p * PC,
                                          nblk, f"m{reg}")
                        st = slabp.tile([128, nblk, 128], dt.bfloat16,
                                        tag=f"s{reg}", name=f"s{reg}")
                        nc.vector.tensor_tensor(
                            st[:],
                            iota_sb[:, None, :].broadcast_to([128, nblk, 128]),
                            dl_sb[reg][:, p * PC:p * PC + nblk, None]
                            .broadcast_to([128, nblk, 128]),
                            op=mybir.AluOpType.is_equal)
                        nc.vector.tensor_tensor(
                            st[:], st[:],
                            nm_sb[reg][:, p * PC:p * PC + nblk, None]
                            .broadcast_to([128, nblk, 128]),
                            op=mybir.AluOpType.mult)
                        pieces[reg] = {p: (mt, st)}  # keep only latest
                    return pieces[reg][p]

                for w in range(W):
                    ps = [[psp.tile([128, 512], dt.float32, tag=f"ps{fh}{q}", name=f"ps{fh}{q}")
                           for q in range(2)] for fh in range(2)]
                    for r in range(R):
                        for reg in range(2):
                            for k in range(int(K[w, r, reg])):
                                ch = int(chunk_of[w, r, reg]) + k
                                p, b = divmod(ch, PC)
                                mt, st = get_piece(reg, p)
                                for fh in range(2):
                                    nc.tensor.matmul(
                                        ps[fh][r // 4][:, (r % 4) * 128:(r % 4) * 128 + 128],
                                        lhsT=mt[:, b, fh * 128:(fh + 1) * 128],
                                        rhs=st[:, b, :],
                                        start=(reg == 0 and k == 0),
                                        stop=(reg == 1 and k == int(K[w, r, 1]) - 1),
                                    )
                    yw = yp.tile([128, 2048], dt.bfloat16, tag="yw", name="yw")
                    for fh in range(2):
                        for q in range(2):
                            eng = nc.vector if q == 0 else nc.scalar
                            (eng.tensor_copy if q == 0 else eng.copy)(
                                yw[:, (fh * 2 + q) * 512:(fh * 2 + q + 1) * 512],
                                ps[fh][q][:])
                    xt = sp.tile([128, 2, 128], dt.bfloat16, tag="xt", name="xt")
                    for fh in range(2):
                        nc.sync.dma_start(
                            xt[:, fh, :],
                            rootsrc[w * 128:(w + 1) * 128, fh * 128:(fh + 1) * 128],
                            transpose=True)
                    agg = psaggp.tile([128, F], dt.float32, tag="agg", name="agg")
                    for r in range(R):
                        for fh in range(2):
                            nc.tensor.matmul(
                                agg[:], lhsT=yw[:, (fh * 8 + r) * 128:(fh * 8 + r + 1) * 128],
                                rhs=w_sb[li][:, r * 2 + fh, :],
                                start=(r == 0 and fh == 0), stop=False)
                    for fh in range(2):
                        nc.tensor.matmul(agg[:], lhsT=xt[:, fh, :],
                                         rhs=rt_sb[li][:, fh, :],
                                         start=False, stop=(fh == 1))
                    hf = sp.tile([128, F], dt.float32, tag="hf", name="hf")
                    nc.vector.tensor_tensor(hf[:], agg[:],
                                            b_sb[li][:],
                                            op=mybir.AluOpType.add)
                    hw = sp.tile([128, F], dt.bfloat16, tag="hw", name="hw")
                    nc.scalar.activation(hw[:], hf[:], AF.Relu)
                    nc.sync.dma_start(hout[w * 128:(w + 1) * 128, :], hw[:])

            layer(0, (x16.ap(), x16.ap()[HIBASE:, :]), xsh.ap(), h1b)
            nc.gpsimd.collective_compute(
                "AllGather", mybir.AluOpType.bypass, replica_groups=rg,
                ins=[h1b[0:NC, :].opt()], outs=[h1f[:].opt()])
            layer(1, (h1f[:], h1f[HIBASE:, :]), h1b[:], h2b)

            # triplet projections u, v per node tile
            for w in range(W):
                ht = sp.tile([128, 2, 128], dt.bfloat16, tag="ht", name="ht")
                for fh in range(2):
                    nc.sync.dma_start(
                        ht[:, fh, :],
                        h2b[w * 128:(w + 1) * 128, fh * 128:(fh + 1) * 128],
                        transpose=True)
                psu = psaggp.tile([128, F], dt.float32, tag="psu", name="psu")
                psv = psaggp.tile([128, F], dt.float32, tag="psv", name="psv")
                for fh in range(2):
                    nc.tensor.matmul(psu[:], lhsT=ht[:, fh, :], rhs=wpu_sb[:, fh, :],
                                     start=(fh == 0), stop=(fh == 1))
                    nc.tensor.matmul(psv[:], lhsT=ht[:, fh, :], rhs=wpv_sb[:, fh, :],
                                     start=(fh == 0), stop=(fh == 1))
                uo = sp.tile([128, F], dt.bfloat16, tag="uo", name="uo")
                nc.vector.tensor_tensor(uo[:], psu[:],
                                        bp_sb[:],
                                        op=mybir.AluOpType.add)
                vo = sp.tile([128, F], dt.bfloat16, tag="vo", name="vo")
                nc.scalar.copy(vo[:], psv[:])
                nc.sync.dma_start(uvb[0][w * 128:(w + 1) * 128, :], uo[:])
                nc.sync.dma_start(uvb[1][w * 128:(w + 1) * 128, :], vo[:])
            for i in range(2):
                nc.gpsimd.collective_compute(
                    "AllGather", mybir.AluOpType.bypass, replica_groups=rg,
                    ins=[uvb[i][0:NC, :].opt()], outs=[uvf[i][:].opt()])

            # triplet gather + add, piece by piece (pieces stay inside groups)
            for gi in range(4):
                a = int(gstart[gi]) // 128
                nb = (int(gstart[gi + 1]) - int(gstart[gi])) // 128
                ub = uvf[0][:] if gi < 2 else uvf[0][HIBASE:, :]
                vb = uvf[1][:] if (gi & 1) == 0 else uvf[1][HIBASE:, :]
                for p0 in range(0, nb, PC):
                    blks = min(PC, nb - p0)
                    gu = gather_piece(ub, tui_sb, a + p0, blks, "gu")
                    gv = gather_piece(vb, tvi_sb, a + p0, blks, "gv")
                    ot = msgp.tile([128, blks, F], dt.bfloat16, tag="ot", name="ot")
                    nc.vector.tensor_tensor(
                        ot[:].rearrange("p b o -> p (b o)"),
                        gu[:].rearrange("p b o -> p (b o)"),
                        gv[:].rearrange("p b o -> p (b o)"),
                        op=mybir.AluOpType.add)
                    nc.sync.dma_start(
                        tout[(a + p0) * 128:(a + p0 + blks) * 128, :]
                        .rearrange("(b p) o -> p b o", p=128),
                        ot[:])
    nc.compile()
    return nc


def kernel(**inputs):
    from concourse.bass_utils import run_bass_kernel_spmd

    x = np.asarray(inputs["x"], dtype=np.float32)
    ei = np.asarray(inputs["edge_index"], dtype=np.int64)
    et = np.asarray(inputs["edge_type"], dtype=np.int64)
    src, dst = ei[0], ei[1]
    cnt = np.bincount(dst * R + et, minlength=N * R)
    norm = (1.0 / np.maximum(cnt[dst * R + et], 1)).astype(np.float32)

    K, chunk_of, nchunk, idx_s, dl_s, nm_s = _plan_agg(src, dst, et, norm)
    gstart, tslots, tplans = _plan_trip(src, dst)
    nc = _build(nchunk, K, chunk_of, gstart, tslots)

    x16 = x.astype(BF16)
    xpad = np.zeros((NPAD, F), dtype=BF16)
    w1 = np.asarray(inputs["W1"], np.float32).astype(BF16)
    w2 = np.asarray(inputs["W2"], np.float32).astype(BF16)
    r1 = np.asarray(inputs["root1"], np.float32).astype(BF16)
    r2 = np.asarray(inputs["root2"], np.float32).astype(BF16)
    wp = np.asarray(inputs["Wp"], np.float32)
    b1 = np.tile(np.asarray(inputs["b1"], np.float32).reshape(1, F), (128, 1))
    b2 = np.tile(np.asarray(inputs["b2"], np.float32).reshape(1, F), (128, 1))
    bp = np.tile(np.asarray(inputs["bp"], np.float32).reshape(1, F), (128, 1))

    in_maps = []
    for c in range(NCORES):
        xs = xpad.copy()
        xs[:NC] = x16[c * NC:(c + 1) * NC]
        iu, iv, _ = tplans[c]
        in_maps.append({
            "x16": x16, "xsh": xs,
            "w1": w1, "w2": w2, "r1": r1, "r2": r2,
            "b1": b1, "b2": b2,
            "wpu": wp[:F].astype(BF16), "wpv": wp[F:].astype(BF16), "bp": bp,
            "idx_lo": idx_s[c][0], "idx_hi": idx_s[c][1],
            "dl_lo": dl_s[c][0], "dl_hi": dl_s[c][1],
            "nm_lo": nm_s[c][0], "nm_hi": nm_s[c][1],
            "tui": iu, "tvi": iv,
        })
    import os
    res = None
    if os.environ.get("BASS_KERNEL_TRACE"):
        try:
            res = run_bass_kernel_spmd(nc, in_maps,
                                       core_ids=list(range(NCORES)), trace=True)
        except Exception:
            res = None
    if res is None:
        res = run_bass_kernel_spmd(nc, in_maps, core_ids=list(range(NCORES)))
    global LAST_EXEC_NS, LAST_TRACE
    LAST_EXEC_NS = res.exec_time_ns
    if res.instructions_and_trace is not None:
        LAST_TRACE = res.instructions_and_trace[1]
    out = np.zeros((E, F), dtype=np.float32)
    for c in range(NCORES):
        t = np.asarray(res.results[c]["tout"]).astype(np.float32)
        orig = tplans[c][2]
        valid = orig >= 0
        out[orig[valid]] = t[valid]
    return out


# revision 7
# speedup vs baseline: 1.6276x; 1.1527x over previous
"""RGCN (2-layer, per-(dst,rel) mean aggregation) + triplet projection,
distributed over 8 Trainium2 NeuronCores (one SPMD Bass/Tile program).

Sharding: destination-node ranges (6250 nodes/core). Aggregate-first:
  y[dst,rel] = (1/cnt) * sum_{src} x[src]   built as one-hot "slab" matmuls
  accumulated in PSUM, then agg = sum_r y_r @ W_r + x @ root + b, ReLU,
  AllGather h. Triplet: u = h@Wp[:256]+bp, v = h@Wp[256:] per node,
  AllGather u & v, then out[e] = u[src_e] + v[dst_e] via dma_gather + add.

Gathers use gpsimd dma_gather (1024 rows per instruction, int16 indices).
The int16 limit (32767 < 50000 rows) is handled by splitting edges into lo
(src<32768, table base row 0) and hi (src>=32768, base row 17232) streams.

Edge slots are packed DENSELY: per-(w,r,reg) run sizes are the max edge
count over the 8 cores (no 128-rounding); runs share 128-row chunks. Each
(chunk, run) overlap is one matmul instance whose one-hot slab
(slab[e, dst%128] = norm, zero outside the run via a dl=200 sentinel) is
built on-chip with a single fused tensor_scalar (iota==dl)*norm op.
The instruction stream is identical on all cores; per-core variation lives
in the gather-index / dl / norm input tensors.
"""

import numpy as np
import ml_dtypes

BF16 = ml_dtypes.bfloat16

N, R, F, E, NCORES = 50000, 8, 256, 400000, 8
NC = N // NCORES             # 6250
W = (NC + 127) // 128        # 49 windows/core
NPAD = W * 128               # 6272
SPLIT = 32768
HIBASE = 17232               # hi idx = src - HIBASE (<= 32767)
PC = 8                       # gather piece = 8 chunks = 1024 rows
LAST_EXEC_NS = None
LAST_TRACE = None


def _wrap_idx(a):
    """[slots or n, 128] -> [128, n]: column j holds row j."""
    return np.ascontiguousarray(a.T)


def _wrap16(idx):
    """int16 idx [slots] -> [128, slots//16]: element i at [i%16, i//16],
    replicated across the 8 gpsimd core partition groups."""
    s = len(idx) // 16
    a = np.ascontiguousarray(idx.reshape(s, 16).T)
    return np.tile(a, (8, 1))


def _plan_agg(src, dst, et, norm):
    """Dense instance plan.  Returns:
    nchunk[2]      : chunk-grid length per region
    insts          : list of (w, r, reg, chunk, lo_in_chunk, hi_in_chunk,
                      first, last)  (common across cores)
    idx16 per core per region, dl/nm [128, n_inst] f32 per core
    """
    core = dst // NC
    percore = []
    counts = np.zeros((NCORES, W, R, 2), dtype=np.int64)
    for c in range(NCORES):
        m = np.where(core == c)[0]
        dl = dst[m] - c * NC
        w = dl >> 7
        reg = (src[m] >= SPLIT).astype(np.int64)
        percore.append((m, dl, w, reg))
        key = (w * R + et[m]) * 2 + reg
        counts[c] = np.bincount(key, minlength=W * R * 2).reshape(W, R, 2)
    S = counts.max(axis=0)                       # [W,R,2] common run sizes
    empty = S.sum(axis=2) == 0
    S[:, :, 0][empty] = 1                        # ensure >=1 instance per (w,r)

    # run offsets within each region's dense slot stream
    off = np.zeros((W, R, 2), dtype=np.int64)
    tot = [0, 0]
    for reg in range(2):
        acc = 0
        for w in range(W):
            for r in range(R):
                off[w, r, reg] = acc
                acc += S[w, r, reg]
        tot[reg] = acc
    nchunk = [(-(-tot[reg] // 128)) for reg in range(2)]

    # instance list: per (w, r): reg 0 then reg 1, chunks ascending
    insts = []
    for w in range(W):
        for r in range(R):
            items = []
            for reg in range(2):
                sz = int(S[w, r, reg])
                if sz == 0:
                    continue
                a, bnd = int(off[w, r, reg]), int(off[w, r, reg]) + sz
                for ch in range(a // 128, (bnd - 1) // 128 + 1):
                    lo = max(a, ch * 128) - ch * 128
                    hi = min(bnd, (ch + 1) * 128) - ch * 128
                    items.append([w, r, reg, ch, lo, hi, False, False])
            items[0][6] = True
            items[-1][7] = True
            insts.extend(items)
    n_inst = len(insts)

    # (w, r, reg, chunk) -> instance id, as a dense array
    nch = max(nchunk)
    lut = np.full((W * R * 2 * nch,), -1, dtype=np.int64)
    for j, (w_, r_, reg_, ch, lo, hi, _, _) in enumerate(insts):
        lut[((w_ * R + r_) * 2 + reg_) * nch + ch] = j

    idx_s, dlnm_s = [], []
    for c in range(NCORES):
        m, dl, w, regs = percore[c]
        r = et[m]
        ipair = []
        dlv = np.full((n_inst, 128), 200.0, dtype=np.float32)
        nmv = np.zeros((n_inst, 128), dtype=np.float32)
        for reg in range(2):
            slots = nchunk[reg] * 128
            idx = np.zeros(slots, dtype=np.int32)
            sel = regs == reg
            mm = m[sel]
            order = np.lexsort((dl[sel], r[sel] + R * w[sel]))
            mm = mm[order]
            wsel, rsel, dsel = w[sel][order], r[sel][order], dl[sel][order]
            runkey = wsel * R + rsel
            runstart = off[wsel, rsel, reg]
            o = np.arange(len(mm))
            starts = np.zeros(len(mm), dtype=np.int64)
            b = np.flatnonzero(np.diff(runkey)) + 1
            starts[b] = o[b]
            starts = np.maximum.accumulate(starts)
            pos = runstart + (o - starts)
            idx[pos] = src[mm] - reg * HIBASE
            ipair.append(_wrap16(idx.astype(np.int16)))
            j = lut[((wsel * R + rsel) * 2 + reg) * nch + (pos >> 7)]
            assert (j >= 0).all()
            dlv[j, pos & 127] = dsel & 127
            nmv[j, pos & 127] = norm[mm]
        idx_s.append(ipair)
        dlnm_s.append((_wrap_idx(dlv), _wrap_idx(nmv)))
    return nchunk, insts, idx_s, dlnm_s


def _plan_trip(src, dst):
    EC = E // NCORES
    gsizes = np.zeros((NCORES, 4), dtype=np.int64)
    percore = []
    for c in range(NCORES):
        ids = np.arange(c * EC, (c + 1) * EC)
        g = (src[ids] >= SPLIT) * 2 + (dst[ids] >= SPLIT)
        order = np.argsort(g, kind="stable")
        ids, g = ids[order], g[order]
        percore.append((ids, g))
        gsizes[c] = np.bincount(g, minlength=4)
    gpad = (-(-gsizes.max(axis=0) // 128)) * 128
    gstart = np.concatenate([[0], np.cumsum(gpad)]).astype(np.int64)
    slots = int(gstart[-1])
    plans = []
    for c in range(NCORES):
        ids, g = percore[c]
        iu = np.zeros(slots, dtype=np.int32)
        iv = np.zeros(slots, dtype=np.int32)
        orig = np.full(slots, -1, dtype=np.int64)
        for gi in range(4):
            sel = ids[g == gi]
            a = int(gstart[gi])
            iu[a:a + len(sel)] = src[sel] - (gi >> 1) * HIBASE
            iv[a:a + len(sel)] = dst[sel] - (gi & 1) * HIBASE
            orig[a:a + len(sel)] = sel
        plans.append((_wrap16(iu.astype(np.int16)),
                      _wrap16(iv.astype(np.int16)), orig))
    return gstart, slots, plans


def _build(nchunk, insts, gstart, tslots):
    import concourse.bass as bass
    import concourse.bacc as bacc
    import concourse.mybir as mybir
    import concourse.tile as tile

    dt = mybir.dt
    nc = bacc.Bacc("TRN2", target_bir_lowering=False, debug=False,
                   num_devices=NCORES)
    AF = mybir.ActivationFunctionType
    n_inst = len(insts)

    x16 = nc.dram_tensor("x16", [N, F], dt.bfloat16, kind="ExternalInput")
    xsh = nc.dram_tensor("xsh", [NPAD, F], dt.bfloat16, kind="ExternalInput")
    w1d = nc.dram_tensor("w1", [R, F, F], dt.bfloat16, kind="ExternalInput")
    w2d = nc.dram_tensor("w2", [R, F, F], dt.bfloat16, kind="ExternalInput")
    r1d = nc.dram_tensor("r1", [F, F], dt.bfloat16, kind="ExternalInput")
    r2d = nc.dram_tensor("r2", [F, F], dt.bfloat16, kind="ExternalInput")
    b1d = nc.dram_tensor("b1", [128, F], dt.float32, kind="ExternalInput")
    b2d = nc.dram_tensor("b2", [128, F], dt.float32, kind="ExternalInput")
    wpud = nc.dram_tensor("wpu", [F, F], dt.bfloat16, kind="ExternalInput")
    wpvd = nc.dram_tensor("wpv", [F, F], dt.bfloat16, kind="ExternalInput")
    bpd = nc.dram_tensor("bp", [128, F], dt.float32, kind="ExternalInput")
    ilo_d = nc.dram_tensor("idx_lo", [128, nchunk[0] * 8], dt.int16, kind="ExternalInput")
    ihi_d = nc.dram_tensor("idx_hi", [128, nchunk[1] * 8], dt.int16, kind="ExternalInput")
    dl_d = nc.dram_tensor("dl", [128, n_inst], dt.float32, kind="ExternalInput")
    nm_d = nc.dram_tensor("nm", [128, n_inst], dt.float32, kind="ExternalInput")
    tui_d = nc.dram_tensor("tui", [128, tslots // 16], dt.int16, kind="ExternalInput")
    tvi_d = nc.dram_tensor("tvi", [128, tslots // 16], dt.int16, kind="ExternalInput")
    tout = nc.dram_tensor("tout", [tslots, F], dt.bfloat16, kind="ExternalOutput")

    rg = [list(range(NCORES))]

    with tile.TileContext(nc) as tc:
        with (
            tc.tile_pool(name="const", bufs=1) as cp,
            tc.tile_pool(name="msg", bufs=3) as msgp,
            tc.tile_pool(name="slab", bufs=4) as slabp,
            tc.tile_pool(name="yw", bufs=2) as yp,
            tc.tile_pool(name="small", bufs=4) as sp,
            tc.tile_pool(name="ps", bufs=1, space="PSUM") as psp,
            tc.tile_pool(name="psagg", bufs=1, space="PSUM") as psaggp,
            tc.tile_pool(name="dram", bufs=1, space="DRAM") as dram,
        ):
            w_sb = [cp.tile([128, 16, F], dt.bfloat16, tag=f"w{i}", name=f"w{i}") for i in range(2)]
            nc.sync.dma_start(w_sb[0][:], w1d.ap().rearrange("r (h p) o -> p (r h) o", p=128))
            nc.sync.dma_start(w_sb[1][:], w2d.ap().rearrange("r (h p) o -> p (r h) o", p=128))
            rt_sb = [cp.tile([128, 2, F], dt.bfloat16, tag=f"rt{i}", name=f"rt{i}") for i in range(2)]
            nc.sync.dma_start(rt_sb[0][:], r1d.ap().rearrange("(h p) o -> p h o", p=128))
            nc.sync.dma_start(rt_sb[1][:], r2d.ap().rearrange("(h p) o -> p h o", p=128))
            b_sb = [cp.tile([128, F], dt.float32, tag=f"b{i}", name=f"b{i}") for i in range(2)]
            nc.sync.dma_start(b_sb[0][:], b1d[:])
            nc.sync.dma_start(b_sb[1][:], b2d[:])
            wpu_sb = cp.tile([128, 2, F], dt.bfloat16, tag="wpu", name="wpu")
            wpv_sb = cp.tile([128, 2, F], dt.bfloat16, tag="wpv", name="wpv")
            nc.sync.dma_start(wpu_sb[:], wpud.ap().rearrange("(h p) o -> p h o", p=128))
            nc.sync.dma_start(wpv_sb[:], wpvd.ap().rearrange("(h p) o -> p h o", p=128))
            bp_sb = cp.tile([128, F], dt.float32, tag="bp", name="bp")
            nc.sync.dma_start(bp_sb[:], bpd[:])
            ilo_sb = cp.tile([128, nchunk[0] * 8], dt.int16, tag="ilo", name="ilo")
            ihi_sb = cp.tile([128, nchunk[1] * 8], dt.int16, tag="ihi", name="ihi")
            nc.sync.dma_start(ilo_sb[:], ilo_d[:])
            nc.sync.dma_start(ihi_sb[:], ihi_d[:])
            dl_sb = cp.tile([128, n_inst], dt.float32, tag="dl", name="dl")
            nm_sb = cp.tile([128, n_inst], dt.float32, tag="nm", name="nm")
            nc.sync.dma_start(dl_sb[:], dl_d[:])
            nc.sync.dma_start(nm_sb[:], nm_d[:])
            tui_sb = cp.tile([128, tslots // 16], dt.int16, tag="tui", name="tui")
            tvi_sb = cp.tile([128, tslots // 16], dt.int16, tag="tvi", name="tvi")
            nc.sync.dma_start(tui_sb[:], tui_d[:])
            nc.sync.dma_start(tvi_sb[:], tvi_d[:])
            iota_sb = cp.tile([128, 128], dt.bfloat16, tag="iota", name="iota")
            nc.gpsimd.iota(iota_sb[:], pattern=[[1, 128]], channel_multiplier=0,
                           allow_small_or_imprecise_dtypes=True)

            h1b = dram.tile([NPAD, F], dt.bfloat16, tag="h1b", name="h1b")
            h2b = dram.tile([NPAD, F], dt.bfloat16, tag="h2b", name="h2b")
            h1f = dram.tile([N, F], dt.bfloat16, addr_space="Shared", tag="h1f", name="h1f")
            uvb = [dram.tile([NPAD, F], dt.bfloat16, tag=f"uvb{i}", name=f"uvb{i}") for i in range(2)]
            uvf = [dram.tile([N, F], dt.bfloat16, addr_space="Shared", tag=f"uvf{i}", name=f"uvf{i}")
                   for i in range(2)]

            def gather_piece(table, idx_sb, c0, nblk, tag):
                """Gather chunks [c0, c0+nblk) of a stream into an SBUF tile
                with one dma_gather (1024 rows max)."""
                t = msgp.tile([128, nblk, F], dt.bfloat16, tag=tag, name=tag)
                ni = nblk * 128
                nc.gpsimd.dma_gather(
                    t[:], table, idx_sb[:, c0 * 8:(c0 + nblk) * 8],
                    ni, ni, F)
                return t

            def layer(li, tables, rootsrc, hout):
                idx_sb = (ilo_sb, ihi_sb)
                pieces = [{}, {}]   # region -> piece idx -> msg_tile

                def get_piece(reg, p):
                    if p not in pieces[reg]:
                        nblk = min(PC, nchunk[reg] - p * PC)
                        mt = gather_piece(tables[reg], idx_sb[reg], p * PC,
                                          nblk, f"m{reg}")
                        pieces[reg] = {p: mt}  # keep only latest
                    return pieces[reg][p]

                ii = 0
                for w in range(W):
                    ps = [[psp.tile([128, 512], dt.float32, tag=f"ps{fh}{q}", name=f"ps{fh}{q}")
                           for q in range(2)] for fh in range(2)]
                    while ii < len(insts) and insts[ii][0] == w:
                        _, r, reg, ch, lo, hi, first, last = insts[ii]
                        p, b = divmod(ch, PC)
                        mt = get_piece(reg, p)
                        st = slabp.tile([128, 128], dt.bfloat16, tag="st", name="st")
                        nc.vector.tensor_scalar(
                            st[:], iota_sb[:],
                            dl_sb[:, ii:ii + 1], nm_sb[:, ii:ii + 1],
                            op0=mybir.AluOpType.is_equal,
                            op1=mybir.AluOpType.mult)
                        for fh in range(2):
                            nc.tensor.matmul(
                                ps[fh][r // 4][:, (r % 4) * 128:(r % 4) * 128 + 128],
                                lhsT=mt[:, b, fh * 128:(fh + 1) * 128],
                                rhs=st[:],
                                start=first, stop=last)
                        ii += 1
                    yw = yp.tile([128, 2048], dt.bfloat16, tag="yw", name="yw")
                    for fh in range(2):
                        for q in range(2):
                            eng = nc.vector if q == 0 else nc.scalar
                            (eng.tensor_copy if q == 0 else eng.copy)(
                                yw[:, (fh * 2 + q) * 512:(fh * 2 + q + 1) * 512],
                                ps[fh][q][:])
                    xt = sp.tile([128, 2, 128], dt.bfloat16, tag="xt", name="xt")
                    for fh in range(2):
                        nc.sync.dma_start(
                            xt[:, fh, :],
                            rootsrc[w * 128:(w + 1) * 128, fh * 128:(fh + 1) * 128],
                            transpose=True)
                    agg = psaggp.tile([128, F], dt.float32, tag="agg", name="agg")
                    for r in range(R):
                        for fh in range(2):
                            nc.tensor.matmul(
                                agg[:], lhsT=yw[:, (fh * 8 + r) * 128:(fh * 8 + r + 1) * 128],
                                rhs=w_sb[li][:, r * 2 + fh, :],
                                start=(r == 0 and fh == 0), stop=False)
                    for fh in range(2):
                        nc.tensor.matmul(agg[:], lhsT=xt[:, fh, :],
                                         rhs=rt_sb[li][:, fh, :],
                                         start=False, stop=(fh == 1))
                    hf = sp.tile([128, F], dt.float32, tag="hf", name="hf")
                    nc.vector.tensor_tensor(hf[:], agg[:],
                                            b_sb[li][:],
                                            op=mybir.AluOpType.add)
                    hw = sp.tile([128, F], dt.bfloat16, tag="hw", name="hw")
                    nc.scalar.activation(hw[:], hf[:], AF.Relu)
                    nc.sync.dma_start(hout[w * 128:(w + 1) * 128, :], hw[:])

            layer(0, (x16.ap(), x16.ap()[HIBASE:, :]), xsh.ap(), h1b)
            nc.gpsimd.collective_compute(
                "AllGather", mybir.AluOpType.bypass, replica_groups=rg,
                ins=[h1b[0:NC, :].opt()], outs=[h1f[:].opt()])
            layer(1, (h1f[:], h1f[HIBASE:, :]), h1b[:], h2b)

            # triplet projections u, v per node tile
            for w in range(W):
                ht = sp.tile([128, 2, 128], dt.bfloat16, tag="ht", name="ht")
                for fh in range(2):
                    nc.sync.dma_start(
                        ht[:, fh, :],
                        h2b[w * 128:(w + 1) * 128, fh * 128:(fh + 1) * 128],
                        transpose=True)
                psu = psaggp.tile([128, F], dt.float32, tag="psu", name="psu")
                psv = psaggp.tile([128, F], dt.float32, tag="psv", name="psv")
                for fh in range(2):
                    nc.tensor.matmul(psu[:], lhsT=ht[:, fh, :], rhs=wpu_sb[:, fh, :],
                                     start=(fh == 0), stop=(fh == 1))
                    nc.tensor.matmul(psv[:], lhsT=ht[:, fh, :], rhs=wpv_sb[:, fh, :],
                                     start=(fh == 0), stop=(fh == 1))
                uo = sp.tile([128, F], dt.bfloat16, tag="uo", name="uo")
                nc.vector.tensor_tensor(uo[:], psu[:],
                                        bp_sb[:],
                                        op=mybir.AluOpType.add)
                vo = sp.tile([128, F], dt.bfloat16, tag="vo", name="vo")
                nc.scalar.copy(vo[:], psv[:])
                nc.sync.dma_start(uvb[0][w * 128:(w + 1) * 128, :], uo[:])
                nc.sync.dma_start(uvb[1][w * 128:(w + 1) * 128, :], vo[:])
            for i in range(2):
                nc.gpsimd.collective_compute(
                    "AllGather", mybir.AluOpType.bypass, replica_groups=rg,
                    ins=[uvb[i][0:NC, :].opt()], outs=[uvf[i][:].opt()])

            # triplet gather + add, piece by piece (pieces stay inside groups)
            for gi in range(4):
                a = int(gstart[gi]) // 128
                nb = (int(gstart[gi + 1]) - int(gstart[gi])) // 128
                ub = uvf[0][:] if gi < 2 else uvf[0][HIBASE:, :]
                vb = uvf[1][:] if (gi & 1) == 0 else uvf[1][HIBASE:, :]
                for p0 in range(0, nb, PC):
                    blks = min(PC, nb - p0)
                    gu = gather_piece(ub, tui_sb, a + p0, blks, "gu")
                    gv = gather_piece(vb, tvi_sb, a + p0, blks, "gv")
                    ot = msgp.tile([128, blks, F], dt.bfloat16, tag="ot", name="ot")
                    nc.vector.tensor_tensor(
                        ot[:].rearrange("p b o -> p (b o)"),
                        gu[:].rearrange("p b o -> p (b o)"),
                        gv[:].rearrange("p b o -> p (b o)"),
                        op=mybir.AluOpType.add)
                    nc.sync.dma_start(
                        tout[(a + p0) * 128:(a + p0 + blks) * 128, :]
                        .rearrange("(b p) o -> p b o", p=128),
                        ot[:])
    nc.compile()
    return nc


def kernel(**inputs):
    from concourse.bass_utils import run_bass_kernel_spmd

    x = np.asarray(inputs["x"], dtype=np.float32)
    ei = np.asarray(inputs["edge_index"], dtype=np.int64)
    et = np.asarray(inputs["edge_type"], dtype=np.int64)
    src, dst = ei[0], ei[1]
    cnt = np.bincount(dst * R + et, minlength=N * R)
    norm = (1.0 / np.maximum(cnt[dst * R + et], 1)).astype(np.float32)

    import time as _t
    _t0 = _t.time()
    nchunk, insts, idx_s, dlnm_s = _plan_agg(src, dst, et, norm)
    gstart, tslots, tplans = _plan_trip(src, dst)
    print(f"[kernel] plan done {_t.time()-_t0:.1f}s", flush=True)
    nc = _build(nchunk, insts, gstart, tslots)
    print(f"[kernel] build+compile done {_t.time()-_t0:.1f}s", flush=True)

    x16 = x.astype(BF16)
    xpad = np.zeros((NPAD, F), dtype=BF16)
    w1 = np.asarray(inputs["W1"], np.float32).astype(BF16)
    w2 = np.asarray(inputs["W2"], np.float32).astype(BF16)
    r1 = np.asarray(inputs["root1"], np.float32).astype(BF16)
    r2 = np.asarray(inputs["root2"], np.float32).astype(BF16)
    wp = np.asarray(inputs["Wp"], np.float32)
    b1 = np.tile(np.asarray(inputs["b1"], np.float32).reshape(1, F), (128, 1))
    b2 = np.tile(np.asarray(inputs["b2"], np.float32).reshape(1, F), (128, 1))
    bp = np.tile(np.asarray(inputs["bp"], np.float32).reshape(1, F), (128, 1))

    in_maps = []
    for c in range(NCORES):
        xs = xpad.copy()
        xs[:NC] = x16[c * NC:(c + 1) * NC]
        iu, iv, _ = tplans[c]
        in_maps.append({
            "x16": x16, "xsh": xs,
            "w1": w1, "w2": w2, "r1": r1, "r2": r2,
            "b1": b1, "b2": b2,
            "wpu": wp[:F].astype(BF16), "wpv": wp[F:].astype(BF16), "bp": bp,
            "idx_lo": idx_s[c][0], "idx_hi": idx_s[c][1],
            "dl": dlnm_s[c][0], "nm": dlnm_s[c][1],
            "tui": iu, "tvi": iv,
        })
    import os
    res = None
    if os.environ.get("BASS_KERNEL_TRACE"):
        try:
            res = run_bass_kernel_spmd(nc, in_maps,
                                       core_ids=list(range(NCORES)), trace=True)
        except Exception:
            res = None
    if res is None:
        res = run_bass_kernel_spmd(nc, in_maps, core_ids=list(range(NCORES)))
    global LAST_EXEC_NS, LAST_TRACE
    LAST_EXEC_NS = res.exec_time_ns
    if res.instructions_and_trace is not None:
        LAST_TRACE = res.instructions_and_trace[1]
    out = np.zeros((E, F), dtype=np.float32)
    for c in range(NCORES):
        t = np.asarray(res.results[c]["tout"]).astype(np.float32)
        orig = tplans[c][2]
        valid = orig >= 0
        out[orig[valid]] = t[valid]
    return out
